# revision 3
# baseline (speedup 1.0000x reference)
"""Trainium2 Bass kernel for 5x5 patch extraction (ZeroPadding2D + gather).

Full input:  images [8, 128, 128, 32] f32
Full output: [8, 128, 128, 800] f32 where
  out[b, i, j, ki*160 + kj*32 + c] = images_padded[b, i+ki, j+kj, c]
  (spatial zero-padding of 2 on each side).

Sharding: data-parallel over batch; core b handles image b; zero
cross-core communication.

Per-core program: stage the image in SBUF as [128 partitions, 4224]
(row i on partition i, 2 zero-columns of padding each side). For each
ki, the (kj, c) block of 160 output floats at (i, j) is a contiguous
sliding window of padded row i+ki-2 starting at element j*32, so one
DMA per ki writes the whole [i, j, 160] block via an
overlapping-window source AP. Row borders are zero-filled from a zero
tile on the second HWDGE queue.

Perf notes (measured on TRN2):
- The HWDGE splits one DMA across n = (largest divisor of the outer
  AP count <= 16) SDMA engines. Odd outer counts (127) pin the whole
  transfer to ONE engine (~20 GB/s); 126 -> 14 engines; 128 -> 16.
  So the 127-row slabs are emitted as 126+1 rows.
- Each DMA gets its own completion semaphore (HWDGE ring-management
  requires <= 1 outstanding DMA per semaphore).
- Concurrent writes to overlapping DRAM ranges from multiple DMAs can
  wedge the device -- all writes here are disjoint.
"""

from contextlib import ExitStack

import numpy as np

import concourse.bass as bass
import concourse.bacc as bacc
import concourse.mybir as mybir
from concourse.bass_utils import run_bass_kernel_spmd

K = 5
H = W = 128
C = 32
B = 8
PAD = (K - 1) // 2  # 2
KC = K * C  # 160
FREE = (W + 2 * PAD) * C  # 4224

_NC_CACHE = {}


def _build_nc():
    nc = bacc.Bacc("TRN2", target_bir_lowering=False, debug=False)
    images = nc.dram_tensor(
        "images", [H, W * C], mybir.dt.float32, kind="ExternalInput"
    )
    out = nc.dram_tensor(
        "out", [H, W, K * K * C], mybir.dt.float32, kind="ExternalOutput"
    )

    with ExitStack() as stack:
        img = stack.enter_context(
            nc.sbuf_tensor("img", [128, FREE], mybir.dt.float32)
        )
        zt = stack.enter_context(nc.sbuf_tensor("zt", [128, KC], mybir.dt.float32))
        s_ms = stack.enter_context(nc.semaphore("s_ms"))
        s_load = stack.enter_context(nc.semaphore("s_load"))
        sA = [stack.enter_context(nc.semaphore(f"sA{i}")) for i in range(8)]
        sZ = [stack.enter_context(nc.semaphore(f"sZ{i}")) for i in range(6)]
        block = stack.enter_context(nc.Block())

        base = img[:, :]
        pstep = base.ap[0][0]  # backing-row size in elements (4224)

        @block.vector
        def _(vector):
            vector.memset(img[:, 0 : PAD * C], 0.0).then_inc(s_ms, 1)
            vector.memset(img[:, FREE - PAD * C : FREE], 0.0).then_inc(s_ms, 1)
            vector.memset(zt[:, :], 0.0).then_inc(s_ms, 1)

        @block.sync
        def _(sync):
            sync.dma_start(
                img[:, PAD * C : FREE - PAD * C], images.ap()
            ).then_inc(s_load, 16)
            sync.wait_ge(s_load, 16)
            sync.wait_ge(s_ms, 2)
            n_dma = 0
            for ki in range(K):
                di = ki - PAD
                i0 = max(0, -di)
                n_i = H - abs(di)
                p0 = max(0, di)
                chunks = (
                    [(0, n_i)] if n_i % 2 == 0 else [(0, n_i - 1), (n_i - 1, 1)]
                )
                for off, cnt in chunks:
                    src = bass.AP(
                        base.tensor,
                        base.offset + (p0 + off) * pstep,
                        [[pstep, cnt], [C, W], [1, KC]],
                    )
                    dst = out[
                        i0 + off : i0 + off + cnt, :, ki * KC : (ki + 1) * KC
                    ]
                    sync.dma_start(dst, src).then_inc(sA[n_dma], 16)
                    n_dma += 1
            for i in range(n_dma):
                sync.wait_ge(sA[i], 16)

        @block.scalar
        def _(scalar):
            scalar.wait_ge(s_ms, 3)
            nz = 0
            for ki in range(K):
                di = ki - PAD
                i0 = max(0, -di)
                n_i = H - abs(di)
                for i_bad in list(range(0, i0)) + list(range(i0 + n_i, H)):
                    scalar.dma_start(
                        out[i_bad, :, ki * KC : (ki + 1) * KC], zt[:, :]
                    ).then_inc(sZ[nz], 16)
                    nz += 1
            for i in range(nz):
                scalar.wait_ge(sZ[i], 16)

    nc.compile()
    return nc


def _get_nc():
    if "nc" not in _NC_CACHE:
        _NC_CACHE["nc"] = _build_nc()
    return _NC_CACHE["nc"]


def run(images: np.ndarray, trace: bool = False, tmpdir=None):
    """Run on 8 cores. Returns (output [8,128,128,800], BassKernelResults)."""
    images = np.ascontiguousarray(np.asarray(images, dtype=np.float32))
    assert images.shape == (B, H, W, C), images.shape
    nc = _get_nc()
    in_maps = [{"images": images[b].reshape(H, W * C)} for b in range(B)]
    last_err = None
    for attempt in range(3):
        try:
            res = run_bass_kernel_spmd(
                nc, in_maps, core_ids=list(range(B)), trace=trace, tmpdir=tmpdir
            )
            break
        except Exception as e:  # transient NRT device errors observed rarely
            last_err = e
            import time as _time

            _time.sleep(2.0 * (attempt + 1))
    else:
        raise last_err
    out = np.stack([res.results[b]["out"] for b in range(B)], axis=0)
    return out.reshape(B, H, W, K * K * C), res


def kernel(images: np.ndarray) -> np.ndarray:
    out, _ = run(images)
    return out


# revision 4
# speedup vs baseline: 1.2391x; 1.2391x over previous
"""Trainium2 Bass kernel for 5x5 patch extraction (ZeroPadding2D + gather).

Full input:  images [8, 128, 128, 32] f32
Full output: [8, 128, 128, 800] f32 where
  out[b, i, j, ki*160 + kj*32 + c] = images_padded[b, i+ki, j+kj, c]
  (spatial zero-padding of 2 on each side).

Sharding: data-parallel over batch; core b handles image b; zero
cross-core communication. The per-core input is padded host-side with
2 zero rows top/bottom ([132, 4096]) so row-shifted SBUF copies of the
image can be loaded entirely in-bounds.

Per-core program (full-materialization pipeline):
1. One DRAM load, split into 4 column pieces, fills
   img5[p, ki*4224 + col] = padded[p+ki, col] -- five row-shifted
   copies of the image, so output row i's whole 5x5 patch band lives
   on partition i. Column pads are memset to zero; row borders are
   zero via the host padding.
2. DVE builds contiguous 800-float output records
   staged[p, jj*800 + ki*160 + kjc] = img5[p, ki*4224 + (j0+jj)*32 + kjc]
   in j-chunks of 8 (double-buffered). DVE only -- GpSimd shares SBUF
   ports with DVE and halves the copy rate if used concurrently.
3. Per chunk, one DMA writes staged records to DRAM with 3200-byte
   contiguous descriptors (outer count 128 -> 16-way SDMA engine
   split, ~366+ GB/s). Chunk q's staging only waits for the load piece
   covering its source columns, so the replica load overlaps the
   output-write stream.

Hardware findings baked in (measured on TRN2):
- The HWDGE splits one DMA across n = (largest divisor of the outer
  AP count <= 16) SDMA engines; odd outer counts pin the whole
  transfer to ONE engine (~20 GB/s). All DMAs here use outer=128.
- Each DMA gets its own completion semaphore (HWDGE ring management
  allows <= 1 outstanding DMA per semaphore, <= 32 DMA semaphores).
- Concurrent DMA writes to overlapping DRAM ranges can wedge the
  device; all writes here are disjoint.
"""

from contextlib import ExitStack

import numpy as np

import concourse.bass as bass
import concourse.bacc as bacc
import concourse.mybir as mybir
from concourse.bass_utils import run_bass_kernel_spmd

K = 5
H = W = 128
C = 32
B = 8
PAD = (K - 1) // 2  # 2
KC = K * C  # 160
ROW = W * C  # 4096
TROW = (W + 2 * PAD) * C  # 4224
JC = 8  # j-chunk size
NQ = W // JC  # 16 chunks
REC = K * K * C  # 800
STG = JC * REC  # 6400 staged elems per partition per chunk
NPIECE = 4
PW = TROW // NPIECE  # 1056 padded cols per load piece

_NC_CACHE = {}


def _build_nc():
    nc = bacc.Bacc("TRN2", target_bir_lowering=False, debug=False)
    images = nc.dram_tensor(
        "images", [H + 2 * PAD, ROW], mybir.dt.float32, kind="ExternalInput"
    )
    out = nc.dram_tensor(
        "out", [H, W, REC], mybir.dt.float32, kind="ExternalOutput"
    )

    with ExitStack() as stack:
        img5 = stack.enter_context(
            nc.sbuf_tensor("img5", [128, K * TROW], mybir.dt.float32)
        )
        stg = [
            stack.enter_context(
                nc.sbuf_tensor(f"stg{b}", [128, STG], mybir.dt.float32)
            )
            for b in range(2)
        ]
        s_ms = stack.enter_context(nc.semaphore("s_ms"))
        s_load = [
            stack.enter_context(nc.semaphore(f"s_load{t}")) for t in range(NPIECE)
        ]
        sv = [stack.enter_context(nc.semaphore(f"sv{q}")) for q in range(NQ)]
        sd = [stack.enter_context(nc.semaphore(f"sd{q}")) for q in range(NQ)]
        block = stack.enter_context(nc.Block())

        b5 = img5[:, :]
        p5 = b5.ap[0][0]
        bs = [t[:, :] for t in stg]
        ps = [b.ap[0][0] for b in bs]

        def piece_for_chunk(q):
            hi_col = q * JC * C + JC * C + KC - 1
            return min(NPIECE - 1, hi_col // PW)

        @block.vector
        def _(vector):
            vector.memset(
                bass.AP(b5.tensor, b5.offset, [[p5, 128], [TROW, K], [1, PAD * C]]),
                0.0,
            ).then_inc(s_ms, 1)
            vector.memset(
                bass.AP(
                    b5.tensor,
                    b5.offset + TROW - PAD * C,
                    [[p5, 128], [TROW, K], [1, PAD * C]],
                ),
                0.0,
            ).then_inc(s_ms, 1)
            for q in range(NQ):
                vector.wait_ge(s_load[piece_for_chunk(q)], 16)
                if q >= 2:
                    vector.wait_ge(sd[q - 2], 16)
                buf = q % 2
                j0 = q * JC
                for ki in range(K):
                    src = bass.AP(
                        b5.tensor,
                        b5.offset + ki * TROW + j0 * C,
                        [[p5, 128], [C, JC], [1, KC]],
                    )
                    dst = bass.AP(
                        bs[buf].tensor,
                        bs[buf].offset + ki * KC,
                        [[ps[buf], 128], [REC, JC], [1, KC]],
                    )
                    ins = vector.tensor_copy(dst, src)
                    if ki == K - 1:
                        ins.then_inc(sv[q], 1)

        @block.sync
        def _(sync):
            sync.wait_ge(s_ms, 2)
            for t in range(NPIECE):
                c0 = max(t * PW, PAD * C)
                c1 = min((t + 1) * PW, TROW - PAD * C)
                wd = c1 - c0
                dst = bass.AP(
                    b5.tensor, b5.offset + c0, [[p5, 128], [TROW, K], [1, wd]]
                )
                src = bass.AP(
                    images, c0 - PAD * C, [[ROW, 128], [ROW, K], [1, wd]]
                )
                sync.dma_start(dst, src).then_inc(s_load[t], 16)
            for q in range(NQ):
                buf = q % 2
                j0 = q * JC
                sync.wait_ge(sv[q], 1)
                src = bass.AP(
                    bs[buf].tensor,
                    bs[buf].offset,
                    [[ps[buf], 128], [REC, JC], [1, REC]],
                )
                dstd = bass.AP(
                    out, j0 * REC, [[W * REC, 128], [REC, JC], [1, REC]]
                )
                sync.dma_start(dstd, src).then_inc(sd[q], 16)
            for q in range(NQ):
                sync.wait_ge(sd[q], 16)

    nc.compile()
    return nc


def _get_nc():
    if "nc" not in _NC_CACHE:
        _NC_CACHE["nc"] = _build_nc()
    return _NC_CACHE["nc"]


def run(images: np.ndarray, trace: bool = False, tmpdir=None):
    """Run on 8 cores. Returns (output [8,128,128,800], BassKernelResults)."""
    images = np.ascontiguousarray(np.asarray(images, dtype=np.float32))
    assert images.shape == (B, H, W, C), images.shape
    nc = _get_nc()
    in_maps = [
        {
            "images": np.pad(
                images[b].reshape(H, ROW), ((PAD, PAD), (0, 0))
            )
        }
        for b in range(B)
    ]
    last_err = None
    for attempt in range(3):
        try:
            res = run_bass_kernel_spmd(
                nc, in_maps, core_ids=list(range(B)), trace=trace, tmpdir=tmpdir
            )
            break
        except Exception as e:  # transient NRT device errors observed rarely
            last_err = e
            import time as _time

            _time.sleep(2.0 * (attempt + 1))
    else:
        raise last_err
    out = np.stack([res.results[b]["out"] for b in range(B)], axis=0)
    return out.reshape(B, H, W, REC), res


def kernel(images: np.ndarray) -> np.ndarray:
    out, _ = run(images)
    return out
